# revision 5
# baseline (speedup 1.0000x reference)
"""Trainium2 Bass kernel for nn_AttentionCritic (MAAC-style attention critic).

Data-parallel over the batch axis across 8 NeuronCores.  Per core:
  - dense encoder / projection / critic matmuls on TensorE (bf16,
    feature-major activations)
  - the per-batch-element 8x8 cross-agent attention on VectorE
    (broadcast multiplies + in-place tree reductions, batch on partitions)
  - activations / PSUM evacuation on ScalarE (Lrelu / Exp / copies)

kernel(**inputs) takes the full-size float32 inputs of reference.setup_inputs()
and returns (qb [N,B,1] f32, regs [N] f32) matching reference.reference().
"""
import numpy as np
import ml_dtypes
from contextlib import ExitStack

import concourse.bass as bass
import concourse.mybir as mybir
import concourse.tile as tile
from concourse import bacc
from concourse.bass_utils import run_bass_kernel_spmd
from concourse.masks import make_identity

bf16 = mybir.dt.bfloat16
f32 = mybir.dt.float32
AF = mybir.ActivationFunctionType
ALU = mybir.AluOpType
AX = mybir.AxisListType

# dims (hardcoded per problem spec)
N = 8          # agents
B = 16384      # global batch
S = 128        # state dim
SA = 160       # state+action dim
H = 256        # hidden
KH = 4         # heads
AD = 64        # attend dim per head
KD = KH * AD   # 256
NCORES = 8
SCALE = float(np.sqrt(AD))  # 8.0

BT = 256       # batch tile (columns per matmul)
PC = 128       # partitions per batch chunk


def build(b_local: int):
    """Build the per-core Bass program for a batch shard of size b_local."""
    n_tiles = b_local // BT
    n_bc = BT // PC

    nc = bacc.Bacc("TRN2", target_bir_lowering=False, debug=False)

    saT = nc.dram_tensor("saT", [N, SA, b_local], bf16, kind="ExternalInput").ap()
    sT = nc.dram_tensor("sT", [N, S, b_local], bf16, kind="ExternalInput").ap()
    Wenc = nc.dram_tensor("Wenc", [SA, H], bf16, kind="ExternalInput").ap()
    Ws = nc.dram_tensor("Ws", [S, H], bf16, kind="ExternalInput").ap()
    Wq = nc.dram_tensor("Wq", [H, KD], bf16, kind="ExternalInput").ap()
    Wk = nc.dram_tensor("Wk", [H, KD], bf16, kind="ExternalInput").ap()
    Wvx = nc.dram_tensor("Wvx", [H + 1, KD], bf16, kind="ExternalInput").ap()
    Wc1 = nc.dram_tensor("Wc1", [2 * H, H], bf16, kind="ExternalInput").ap()
    Wc2 = nc.dram_tensor("Wc2", [H, 1], bf16, kind="ExternalInput").ap()
    Wb1 = nc.dram_tensor("Wb1", [H, H], bf16, kind="ExternalInput").ap()
    Wb2n = nc.dram_tensor("Wb2n", [H, 1], bf16, kind="ExternalInput").ap()
    Benc = nc.dram_tensor("Benc", [H], f32, kind="ExternalInput").ap()
    Bs = nc.dram_tensor("Bs", [H], f32, kind="ExternalInput").ap()
    Bb1 = nc.dram_tensor("Bb1", [H], f32, kind="ExternalInput").ap()
    Bc1 = nc.dram_tensor("Bc1", [H], f32, kind="ExternalInput").ap()
    Bqb = nc.dram_tensor("Bqb", [1], f32, kind="ExternalInput").ap()

    out = nc.dram_tensor("out", [N, b_local + 1], f32, kind="ExternalOutput").ap()

    with tile.TileContext(nc) as tc, ExitStack() as ctx:
        wpool = ctx.enter_context(tc.tile_pool(name="weights", bufs=1))
        inpool = ctx.enter_context(tc.tile_pool(name="inputs", bufs=3))
        encsa_pool = ctx.enter_context(tc.tile_pool(name="encsa", bufs=2))
        encs_pool = ctx.enter_context(tc.tile_pool(name="encs", bufs=2))
        hb_pool = ctx.enter_context(tc.tile_pool(name="hb", bufs=2))
        q_pool = ctx.enter_context(tc.tile_pool(name="qkv", bufs=6))
        prod_pool = ctx.enter_context(tc.tile_pool(name="prod", bufs=1))
        attn_pool = ctx.enter_context(tc.tile_pool(name="attn", bufs=2))
        other_pool = ctx.enter_context(tc.tile_pool(name="other", bufs=2))
        otherT_pool = ctx.enter_context(tc.tile_pool(name="otherT", bufs=2))
        hc1_pool = ctx.enter_context(tc.tile_pool(name="hc1", bufs=2))
        qb_pool = ctx.enter_context(tc.tile_pool(name="qb", bufs=3))
        acc_pool = ctx.enter_context(tc.tile_pool(name="acc", bufs=1))
        mm_psum = ctx.enter_context(tc.tile_pool(name="mmps", bufs=3, space="PSUM"))
        tr_psum = ctx.enter_context(tc.tile_pool(name="trps", bufs=2, space="PSUM"))
        qb_psum = ctx.enter_context(tc.tile_pool(name="qbps", bufs=2, space="PSUM"))
        rg_psum = ctx.enter_context(tc.tile_pool(name="rgps", bufs=1, space="PSUM"))

        # ---------------- static tiles ----------------
        wenc0 = wpool.tile([128, H], bf16)
        wenc1 = wpool.tile([SA - 128, H], bf16)
        nc.sync.dma_start(wenc0[:], Wenc[0:128, :])
        nc.sync.dma_start(wenc1[:], Wenc[128:SA, :])
        ws0 = wpool.tile([128, H], bf16)
        nc.sync.dma_start(ws0[:], Ws)
        wq = [wpool.tile([128, KD], bf16, tag=f"wq{c}", name=f"wq{c}") for c in range(2)]
        wk = [wpool.tile([128, KD], bf16, tag=f"wk{c}", name=f"wk{c}") for c in range(2)]
        wv = [wpool.tile([128, KD], bf16, tag=f"wv{c}", name=f"wv{c}") for c in range(2)]
        for c in range(2):
            nc.sync.dma_start(wq[c][:], Wq[c * 128:(c + 1) * 128, :])
            nc.sync.dma_start(wk[c][:], Wk[c * 128:(c + 1) * 128, :])
            nc.sync.dma_start(wv[c][:], Wvx[c * 128:(c + 1) * 128, :])
        wvb = wpool.tile([1, KD], bf16)
        nc.sync.dma_start(wvb[:], Wvx[H:H + 1, :])
        wc1 = [wpool.tile([128, H], bf16, tag=f"wc1{c}", name=f"wc1{c}") for c in range(4)]
        for c in range(4):
            nc.sync.dma_start(wc1[c][:], Wc1[c * 128:(c + 1) * 128, :])
        wb1 = [wpool.tile([128, H], bf16, tag=f"wb1{c}", name=f"wb1{c}") for c in range(2)]
        for c in range(2):
            nc.sync.dma_start(wb1[c][:], Wb1[c * 128:(c + 1) * 128, :])
        wc2 = [wpool.tile([128, 1], bf16, tag=f"wc2{c}", name=f"wc2{c}") for c in range(2)]
        wb2n = [wpool.tile([128, 1], bf16, tag=f"wb2n{c}", name=f"wb2n{c}") for c in range(2)]
        for c in range(2):
            nc.sync.dma_start(wc2[c][:], Wc2[c * 128:(c + 1) * 128, :])
            nc.sync.dma_start(wb2n[c][:], Wb2n[c * 128:(c + 1) * 128, :])

        benc = wpool.tile([128, 2], f32)
        bs = wpool.tile([128, 2], f32)
        bb1 = wpool.tile([128, 2], f32)
        bc1 = wpool.tile([128, 2], f32)
        for c in range(2):
            nc.sync.dma_start(benc[:, c:c + 1], Benc[c * 128:(c + 1) * 128].unsqueeze(1))
            nc.sync.dma_start(bs[:, c:c + 1], Bs[c * 128:(c + 1) * 128].unsqueeze(1))
            nc.sync.dma_start(bb1[:, c:c + 1], Bb1[c * 128:(c + 1) * 128].unsqueeze(1))
            nc.sync.dma_start(bc1[:, c:c + 1], Bc1[c * 128:(c + 1) * 128].unsqueeze(1))
        bqb = wpool.tile([1, 1], f32)
        nc.sync.dma_start(bqb[:], Bqb.unsqueeze(1))

        ones_b = wpool.tile([1, 128], bf16)
        nc.vector.memset(ones_b[:], 1.0)
        onesf = wpool.tile([128, 1], f32)
        nc.vector.memset(onesf[:], 1.0)
        ident = wpool.tile([128, 128], f32)
        make_identity(nc, ident[:])

        regs_acc = acc_pool.tile([128, N], f32)
        nc.vector.memset(regs_acc[:], 0.0)

        # ---------------- main loop over batch tiles ----------------
        for t in range(n_tiles):
            b0 = t * BT
            encsa = encsa_pool.tile([128, 2, N, BT], bf16)
            hb = hb_pool.tile([128, 2, N, BT], bf16)
            q_cs = [q_pool.tile([128, N, KH, AD], bf16, tag="qc", name="qc") for _ in range(n_bc)]
            k_cs = [q_pool.tile([128, N, KH, AD], bf16, tag="kc", name="kc") for _ in range(n_bc)]
            v_cs = [q_pool.tile([128, KH, AD, N], bf16, tag="vc", name="vc") for _ in range(n_bc)]

            for n in range(N):
                sa0 = inpool.tile([128, BT], bf16, tag="sa0")
                sa1 = inpool.tile([SA - 128, BT], bf16, tag="sa1")
                s0 = inpool.tile([128, BT], bf16, tag="s0")
                nc.sync.dma_start(sa0[:], saT[n, 0:128, b0:b0 + BT])
                nc.sync.dma_start(sa1[:], saT[n, 128:SA, b0:b0 + BT])
                nc.sync.dma_start(s0[:], sT[n, :, b0:b0 + BT])

                encs = encs_pool.tile([128, 2, BT], bf16)
                for hc in range(2):
                    hsl = slice(hc * 128, (hc + 1) * 128)
                    ps = mm_psum.tile([128, BT], f32, tag="mm")
                    nc.tensor.matmul(ps[:], wenc0[:, hsl], sa0[:], start=True, stop=False)
                    nc.tensor.matmul(ps[:], wenc1[:, hsl], sa1[:], start=False, stop=True)
                    nc.scalar.activation(out=encsa[:, hc, n, :], in_=ps[:],
                                         func=AF.Lrelu, bias=benc[:, hc:hc + 1],
                                         scale=1.0, alpha=0.01)
                    ps2 = mm_psum.tile([128, BT], f32, tag="mm")
                    nc.tensor.matmul(ps2[:], ws0[:, hsl], s0[:], start=True, stop=True)
                    nc.scalar.activation(out=encs[:, hc, :], in_=ps2[:],
                                         func=AF.Lrelu, bias=bs[:, hc:hc + 1],
                                         scale=1.0, alpha=0.01)
                # baseline-head hidden layer
                for hc in range(2):
                    hsl = slice(hc * 128, (hc + 1) * 128)
                    ps = mm_psum.tile([128, BT], f32, tag="mm")
                    nc.tensor.matmul(ps[:], wb1[0][:, hsl], encs[:, 0, :], start=True, stop=False)
                    nc.tensor.matmul(ps[:], wb1[1][:, hsl], encs[:, 1, :], start=False, stop=True)
                    nc.scalar.activation(out=hb[:, hc, n, :], in_=ps[:],
                                         func=AF.Lrelu, bias=bb1[:, hc:hc + 1],
                                         scale=1.0, alpha=0.01)
                # Q / K / V (batch-major)
                for bc in range(n_bc):
                    bsl = slice(bc * PC, (bc + 1) * PC)
                    psq = mm_psum.tile([128, KD], f32, tag="mm")
                    nc.tensor.matmul(psq[:], encs[:, 0, bsl], wq[0][:], start=True, stop=False)
                    nc.tensor.matmul(psq[:], encs[:, 1, bsl], wq[1][:], start=False, stop=True)
                    nc.scalar.copy(q_cs[bc][:, n, :, :].rearrange("p k d -> p (k d)"), psq[:])
                    psk = mm_psum.tile([128, KD], f32, tag="mm")
                    nc.tensor.matmul(psk[:], encsa[:, 0, n, bsl], wk[0][:], start=True, stop=False)
                    nc.tensor.matmul(psk[:], encsa[:, 1, n, bsl], wk[1][:], start=False, stop=True)
                    nc.scalar.copy(k_cs[bc][:, n, :, :].rearrange("p k d -> p (k d)"), psk[:])
                    psv = mm_psum.tile([128, KD], f32, tag="mm")
                    nc.tensor.matmul(psv[:], encsa[:, 0, n, bsl], wv[0][:], start=True, stop=False)
                    nc.tensor.matmul(psv[:], encsa[:, 1, n, bsl], wv[1][:], start=False, stop=False)
                    nc.tensor.matmul(psv[:], ones_b[:], wvb[:], start=False, stop=True)
                    # v layout (k, d, j): strided write at j=n
                    nc.scalar.activation(
                        out=v_cs[bc][:, :, :, n],
                        in_=psv[:].rearrange("p (k d) -> p k d", k=KH),
                        func=AF.Lrelu, scale=1.0, alpha=0.01)

            # ---------------- attention per 128-batch chunk ----------------
            others = []
            for bc in range(n_bc):
                q_c, k_c, v_c = q_cs[bc], k_cs[bc], v_cs[bc]
                prod = prod_pool.tile([128, N, KH, N, AD], bf16, tag="prod")
                for kh in range(KH):
                    q_ap = q_c[:, :, kh, :].unsqueeze(2).broadcast_to([128, N, N, AD])
                    k_ap = k_c[:, :, kh, :].unsqueeze(1).broadcast_to([128, N, N, AD])
                    nc.vector.tensor_tensor(out=prod[:, :, kh, :, :], in0=q_ap,
                                            in1=k_ap, op=ALU.mult)
                # tree-reduce over d (in place)
                w = AD
                while w > 2:
                    h = w // 2
                    nc.vector.tensor_tensor(out=prod[:, :, :, :, 0:h],
                                            in0=prod[:, :, :, :, 0:h],
                                            in1=prod[:, :, :, :, h:w], op=ALU.add)
                    w = h
                L = attn_pool.tile([128, N, KH, N], f32, tag="L")
                nc.vector.tensor_tensor(out=L[:], in0=prod[:, :, :, :, 0:1].squeeze(4),
                                        in1=prod[:, :, :, :, 1:2].squeeze(4), op=ALU.add)
                # regs: zero the diagonal, square, reduce over (k, j)
                lap = L[:]
                diag = bass.AP(tensor=L.tensor, offset=lap.offset,
                               ap=[list(lap.ap[0]), [KH * N + 1, N], [N, KH]])
                nc.vector.memset(diag, 0.0)
                sq = attn_pool.tile([128, N, KH, N], f32, tag="sq")
                nc.vector.tensor_tensor(out=sq[:], in0=L[:], in1=L[:], op=ALU.mult)
                rg = attn_pool.tile([128, N], f32, tag="rg")
                nc.vector.tensor_reduce(out=rg[:],
                                        in_=sq[:].rearrange("p i k j -> p i (k j)"),
                                        axis=AX.X, op=ALU.add)
                nc.vector.tensor_tensor(out=regs_acc[:], in0=regs_acc[:], in1=rg[:],
                                        op=ALU.add)
                # softmax over j (diag -> -8e9; exp(x/8); no max subtraction needed)
                nc.vector.memset(diag, -8.0e9)
                E = attn_pool.tile([128, N, KH, N], f32, tag="E")
                nc.scalar.activation(out=E[:], in_=L[:], func=AF.Exp, scale=1.0 / SCALE)
                den = attn_pool.tile([128, N, KH], f32, tag="den")
                nc.vector.tensor_reduce(out=den[:], in_=E[:], axis=AX.X, op=ALU.add)
                rec = attn_pool.tile([128, N, KH], f32, tag="rec")
                nc.vector.reciprocal(out=rec[:], in_=den[:])
                P = attn_pool.tile([128, N, KH, N], bf16, tag="P")
                nc.vector.tensor_tensor(
                    out=P[:], in0=E[:],
                    in1=rec[:].unsqueeze(3).broadcast_to([128, N, KH, N]), op=ALU.mult)
                # attention output: prod2[(k,i,d,j)] = P[i,k,j] * V[k,d,j]
                prod2 = prod_pool.tile([128, N, KH, AD, N], bf16, tag="prod")
                for kh in range(KH):
                    p_ap = P[:, :, kh, :].unsqueeze(2).broadcast_to([128, N, AD, N])
                    v_ap = v_c[:, kh, :, :].unsqueeze(1).broadcast_to([128, N, AD, N])
                    nc.vector.tensor_tensor(out=prod2[:, :, kh, :, :], in0=p_ap,
                                            in1=v_ap, op=ALU.mult)
                w = N
                while w > 2:
                    h = w // 2
                    nc.vector.tensor_tensor(out=prod2[:, :, :, :, 0:h],
                                            in0=prod2[:, :, :, :, 0:h],
                                            in1=prod2[:, :, :, :, h:w], op=ALU.add)
                    w = h
                other = other_pool.tile([128, N, KH, AD], f32, tag="other")
                nc.vector.tensor_tensor(out=other[:], in0=prod2[:, :, :, :, 0:1].squeeze(4),
                                        in1=prod2[:, :, :, :, 1:2].squeeze(4), op=ALU.add)
                others.append(other)

            # ---------------- critic heads per agent ----------------
            for n in range(N):
                otherT = otherT_pool.tile([128, 2, BT], bf16, tag="otherT")
                for half in range(2):
                    pst = tr_psum.tile([128, n_bc, PC], f32, tag="tr")
                    for bc in range(n_bc):
                        src = others[bc][:, n, 2 * half:2 * half + 2, :]
                        nc.tensor.transpose(pst[:, bc, :], src, ident[:])
                    nc.scalar.copy(otherT[:, half, :], pst[:].rearrange("p c b -> p (c b)"))
                hc1 = hc1_pool.tile([128, 2, BT], bf16)
                for hc in range(2):
                    hsl = slice(hc * 128, (hc + 1) * 128)
                    ps = mm_psum.tile([128, BT], f32, tag="mm")
                    nc.tensor.matmul(ps[:], wc1[0][:, hsl], encsa[:, 0, n, :], start=True, stop=False)
                    nc.tensor.matmul(ps[:], wc1[1][:, hsl], encsa[:, 1, n, :], start=False, stop=False)
                    nc.tensor.matmul(ps[:], wc1[2][:, hsl], otherT[:, 0, :], start=False, stop=False)
                    nc.tensor.matmul(ps[:], wc1[3][:, hsl], otherT[:, 1, :], start=False, stop=True)
                    nc.scalar.activation(out=hc1[:, hc, :], in_=ps[:], func=AF.Lrelu,
                                         bias=bc1[:, hc:hc + 1], scale=1.0, alpha=0.01)
                pq = qb_psum.tile([1, BT], f32, tag="qb")
                nc.tensor.matmul(pq[:], wc2[0][:], hc1[:, 0, :], start=True, stop=False)
                nc.tensor.matmul(pq[:], wc2[1][:], hc1[:, 1, :], start=False, stop=False)
                nc.tensor.matmul(pq[:], wb2n[0][:], hb[:, 0, n, :], start=False, stop=False)
                nc.tensor.matmul(pq[:], wb2n[1][:], hb[:, 1, n, :], start=False, stop=True)
                qb = qb_pool.tile([1, BT], f32)
                nc.scalar.activation(out=qb[:], in_=pq[:], func=AF.Identity,
                                     bias=bqb[:], scale=1.0)
                nc.sync.dma_start(out[n, b0:b0 + BT].unsqueeze(0), qb[:])

        # ---------------- regs epilogue ----------------
        psr = rg_psum.tile([N, 1], f32, tag="regs")
        nc.tensor.matmul(psr[:], regs_acc[:], onesf[:], start=True, stop=True)
        rg_sb = acc_pool.tile([N, 1], f32)
        nc.scalar.copy(rg_sb[:], psr[:])
        nc.sync.dma_start(out[:, b_local:b_local + 1], rg_sb[:])

    nc.compile()
    return nc


# ---------------------------------------------------------------------------
_CACHE = {}


def _get_nc(b_local: int):
    if b_local not in _CACHE:
        _CACHE[b_local] = build(b_local)
    return _CACHE[b_local]


def _to_bf16(x):
    return np.asarray(x, np.float32).astype(ml_dtypes.bfloat16)


def prepare_in_maps(states, sa, W_enc, b_enc, W_s, b_s, W_k, W_q, W_v, b_v,
                    W_c1, b_c1, W_c2, b_c2, W_b1, b_b1, W_b2, b_b2,
                    n_cores=NCORES):
    states = np.asarray(states, np.float32)
    sa = np.asarray(sa, np.float32)
    b = states.shape[1]
    b_local = b // n_cores
    # feature-major inputs (transpose on host; cast to bf16)
    saT = np.ascontiguousarray(sa.transpose(0, 2, 1)).astype(ml_dtypes.bfloat16)
    sT = np.ascontiguousarray(states.transpose(0, 2, 1)).astype(ml_dtypes.bfloat16)
    wq = np.ascontiguousarray(np.asarray(W_q, np.float32).transpose(1, 0, 2).reshape(H, KD))
    wk = np.ascontiguousarray(np.asarray(W_k, np.float32).transpose(1, 0, 2).reshape(H, KD))
    wv = np.asarray(W_v, np.float32).transpose(1, 0, 2).reshape(H, KD)
    wvx = np.ascontiguousarray(np.concatenate([wv, np.asarray(b_v, np.float32).reshape(1, KD)], 0))
    shared = dict(
        Wenc=_to_bf16(W_enc), Ws=_to_bf16(W_s), Wq=_to_bf16(wq), Wk=_to_bf16(wk),
        Wvx=_to_bf16(wvx), Wc1=_to_bf16(W_c1), Wc2=_to_bf16(np.reshape(W_c2, (H, 1))),
        Wb1=_to_bf16(W_b1), Wb2n=_to_bf16(-np.reshape(np.asarray(W_b2, np.float32), (H, 1))),
        Benc=np.asarray(b_enc, np.float32).reshape(H),
        Bs=np.asarray(b_s, np.float32).reshape(H),
        Bb1=np.asarray(b_b1, np.float32).reshape(H),
        Bc1=np.asarray(b_c1, np.float32).reshape(H),
        Bqb=(np.asarray(b_c2, np.float32).reshape(1) - np.asarray(b_b2, np.float32).reshape(1)),
    )
    in_maps = []
    for c in range(n_cores):
        bs_ = slice(c * b_local, (c + 1) * b_local)
        m = dict(shared)
        m["saT"] = np.ascontiguousarray(saT[:, :, bs_])
        m["sT"] = np.ascontiguousarray(sT[:, :, bs_])
        in_maps.append(m)
    return in_maps, b_local


def postprocess(results, b_local, n_cores=NCORES):
    qb = np.concatenate([r["out"][:, :b_local] for r in results], axis=1)
    qb = qb.reshape(N, n_cores * b_local, 1).astype(np.float32)
    raw = np.sum([r["out"][:, b_local].astype(np.float64) for r in results], axis=0)
    regs = (0.001 * raw / (n_cores * b_local * (N - 1))).astype(np.float32)
    return qb, regs


def run(inputs, n_cores=NCORES, **run_kwargs):
    in_maps, b_local = prepare_in_maps(**inputs, n_cores=n_cores)
    nc = _get_nc(b_local)
    res = run_bass_kernel_spmd(nc, in_maps, core_ids=list(range(n_cores)), **run_kwargs)
    return postprocess(res.results, b_local, n_cores), res


def kernel(**inputs):
    (qb, regs), _ = run(inputs)
    return qb, regs
